# revision 10
# baseline (speedup 1.0000x reference)
"""ChannelWiseTCN Trainium2 Bass kernel.

Reference model: 4-layer TCN. Each layer: tied-kernel causal depthwise conv
(K=3, dilation 2^i) -> 1x1 pointwise conv -> BatchNorm(eval) -> ReLU ->
residual add -> ReLU. Final 1x1 conv to 1 channel.

Strategy:
  - Data-parallel over batch: 32 batches / 8 cores = 4 per core; small
    weights replicated.
  - Host-side folding: BN scale/shift folded into pointwise weights/bias.
    Depthwise tap scalar sc_w[i,k] folded into the pointwise weight too, so
    each layer becomes 3 matmuls (one per tap, shifted view of the input
    along the seq dim) accumulated in PSUM, plus bias+ReLU epilogue.
  - Activations stay SBUF-resident: x [64, 16+4096] per batch, h buffers
    [128, 16+4096]; the 16-col zero pad provides the causal-conv history
    (max lookback = (K-1)*d_max = 16).
"""

import os
import numpy as np

import concourse.bass as bass
import concourse.tile as tile
from concourse import bacc, mybir
from concourse.bass_utils import run_bass_kernel_spmd

F32 = mybir.dt.float32
AF = mybir.ActivationFunctionType
ALU = mybir.AluOpType

B, CIN, H, L, K, NL = 32, 64, 128, 4096, 3, 4
EPS = 1e-5
NCORES = 8
BLOC = B // NCORES          # batches per core
PAD = 16                    # max causal lookback: (K-1) * 2^(NL-1) = 16
CH = 512                    # seq chunk (PSUM fp32 free-dim limit)
NCH = L // CH

LAST_RESULTS = None         # stash for test.py (exec_time_ns etc.)
REPLICAS = 1                # timing knob: run the whole TCN R times in one NEFF


def _preprocess(inputs):
    """Fold BN + tap scalars into matmul-ready weights (host, float64)."""
    f64 = {k: np.asarray(v, np.float64) for k, v in inputs.items()}
    s = f64["bn_g"] / np.sqrt(f64["bn_v"] + EPS)                 # (NL,H)
    bias = s * (f64["pw_b"] - f64["bn_m"]) + f64["bn_b"]         # (NL,H)

    sc = f64["sc_w"]                                             # (NL,K)
    W0 = s[0][:, None] * f64["pw_w0"]                            # (H,CIN)
    lhsT0 = np.stack([W0.T * sc[0, k] for k in range(K)])        # (K,CIN,H)
    lhsTr = np.stack([
        np.stack([(s[i][:, None] * f64["pw_w_rest"][i - 1]).T * sc[i, k]
                  for k in range(K)])
        for i in range(1, NL)
    ])                                                           # (NL-1,K,H,H)

    return {
        "lhsT0": lhsT0.astype(np.float32),
        "lhsTr": lhsTr.astype(np.float32),
        "resT": np.ascontiguousarray(f64["res_w"].T).astype(np.float32),   # (CIN,H)
        "outT": np.ascontiguousarray(f64["out_w"].T).astype(np.float32),   # (H,1)
        "bias": bias[:, :, None].astype(np.float32),             # (NL,H,1)
        "res_b": f64["res_b"][:, None].astype(np.float32),       # (H,1)
        "out_b": f64["out_b"][:, None].astype(np.float32),       # (1,1)
    }


def _build():
    """Build the per-core Bass program. Returns nc."""
    nc = bacc.Bacc("TRN2", target_bir_lowering=False, debug=False)

    x_d = nc.dram_tensor("x", [BLOC, CIN, L], F32, kind="ExternalInput").ap()
    lhsT0_d = nc.dram_tensor("lhsT0", [K, CIN, H], F32, kind="ExternalInput").ap()
    lhsTr_d = nc.dram_tensor("lhsTr", [NL - 1, K, H, H], F32, kind="ExternalInput").ap()
    resT_d = nc.dram_tensor("resT", [CIN, H], F32, kind="ExternalInput").ap()
    outT_d = nc.dram_tensor("outT", [H, 1], F32, kind="ExternalInput").ap()
    bias_d = nc.dram_tensor("bias", [NL, H, 1], F32, kind="ExternalInput").ap()
    res_b_d = nc.dram_tensor("res_b", [H, 1], F32, kind="ExternalInput").ap()
    out_b_d = nc.dram_tensor("out_b", [1, 1], F32, kind="ExternalInput").ap()
    y_d = nc.dram_tensor("y", [BLOC, 1, L], F32, kind="ExternalOutput").ap()

    with tile.TileContext(nc) as tc:
        with (
            tc.tile_pool(name="const", bufs=1) as cpool,
            tc.tile_pool(name="xp", bufs=2) as xpool,
            tc.tile_pool(name="hp", bufs=3) as hpool,
            tc.tile_pool(name="ap", bufs=4) as apool,
            tc.tile_pool(name="op", bufs=2) as opool,
            tc.tile_pool(name="psy", bufs=4, space="PSUM") as psy,
            tc.tile_pool(name="psr", bufs=2, space="PSUM") as psr,
            tc.tile_pool(name="pso", bufs=2, space="PSUM") as pso,
        ):
            # ---- load weights/biases once ----
            wt = []  # wt[i][k] lhsT tile
            row = []
            for kk in range(K):
                t = cpool.tile([CIN, H], F32, name=f"w0t_{kk}")
                nc.sync.dma_start(t[:], lhsT0_d[kk])
                row.append(t)
            wt.append(row)
            for i in range(1, NL):
                row = []
                for kk in range(K):
                    t = cpool.tile([H, H], F32, name=f"w{i}t_{kk}")
                    nc.sync.dma_start(t[:], lhsTr_d[i - 1, kk])
                    row.append(t)
                wt.append(row)
            wres = cpool.tile([CIN, H], F32, name="wres")
            nc.sync.dma_start(wres[:], resT_d[:])
            wout = cpool.tile([H, 1], F32, name="wout")
            nc.sync.dma_start(wout[:], outT_d[:])
            btile = []
            for i in range(NL):
                t = cpool.tile([H, 1], F32, name=f"bias_{i}")
                nc.sync.dma_start(t[:], bias_d[i])
                btile.append(t)
            rb = cpool.tile([H, 1], F32, name="res_b")
            nc.sync.dma_start(rb[:], res_b_d[:])
            ob = cpool.tile([1, 1], F32, name="out_b")
            nc.sync.dma_start(ob[:], out_b_d[:])

            # All-engine sync point: later instructions never carry waits on
            # the const-load DMA queues (walrus caps sync waits per inst).
            tc.strict_bb_all_engine_barrier()

            # ---- main loop ----
            for b in [bb for _ in range(REPLICAS) for bb in range(BLOC)]:
                xt = xpool.tile([CIN, PAD + L], F32, name="xt", tag="xt")
                nc.vector.memset(xt[:, :PAD], 0.0)
                nc.sync.dma_start(xt[:, PAD:], x_d[b])

                hprev = xt
                for i in range(NL):
                    d = 1 << i
                    ht = hpool.tile([H, PAD + L], F32, name=f"h{i}", tag="h")
                    nc.vector.memset(ht[:, :PAD], 0.0)
                    for j in range(NCH):
                        c0 = PAD + j * CH
                        yp = psy.tile([H, CH], F32, name="yp", tag="yp")
                        for kk in range(K):
                            off = (K - 1 - kk) * d
                            nc.tensor.matmul(
                                yp[:], wt[i][kk][:],
                                hprev[:, c0 - off: c0 - off + CH],
                                start=(kk == 0), stop=(kk == K - 1),
                            )
                        at = apool.tile([H, CH], F32, name="at", tag="at")
                        nc.scalar.activation(at[:], yp[:], AF.Relu, bias=btile[i][:])
                        dst = ht[:, c0: c0 + CH]
                        if i == 0:
                            rp = psr.tile([H, CH], F32, name="rp", tag="rp")
                            nc.tensor.matmul(rp[:], wres[:], xt[:, c0: c0 + CH],
                                             start=True, stop=True)
                            nc.vector.scalar_tensor_tensor(
                                dst, rp[:], rb[:], at[:], ALU.add, ALU.add)
                            nc.vector.tensor_scalar_max(dst, dst, 0.0)
                        else:
                            # relu(relu(y)+h_prev) == relu(y)+h_prev: both
                            # operands are non-negative post-ReLU.
                            nc.vector.tensor_add(dst, at[:], hprev[:, c0: c0 + CH])
                    hprev = ht

                ot = opool.tile([1, L], F32, name="ot", tag="ot")
                for j in range(NCH):
                    c0 = PAD + j * CH
                    opsum = pso.tile([1, CH], F32, name="opsum", tag="opsum")
                    nc.tensor.matmul(opsum[:], wout[:], hprev[:, c0: c0 + CH],
                                     start=True, stop=True)
                    nc.scalar.activation(ot[:, j * CH:(j + 1) * CH], opsum[:],
                                         AF.Identity, bias=ob[:])
                nc.sync.dma_start(y_d[b], ot[:])

    nc.compile()
    return nc


def _in_maps(inputs):
    w = _preprocess(inputs)
    x = np.ascontiguousarray(np.asarray(inputs["x"], np.float32))
    maps = []
    for c in range(NCORES):
        m = dict(w)
        m["x"] = np.ascontiguousarray(x[c * BLOC:(c + 1) * BLOC])
        maps.append(m)
    return maps


def kernel(**inputs):
    global LAST_RESULTS
    os.environ.setdefault("BASS_NEVER_TRACE", "1")  # no NTFF hook in this env
    nc = _build()
    maps = _in_maps(inputs)
    res = run_bass_kernel_spmd(nc, maps, core_ids=list(range(NCORES)))
    LAST_RESULTS = res
    out = np.concatenate([r["y"] for r in res.results], axis=0)
    return out.astype(np.float32)


def simulate_core(inputs, core=0):
    """CoreSim numeric check of one core's shard (no hardware)."""
    from concourse.bass_interp import CoreSim
    nc = _build()
    maps = _in_maps(inputs)
    sim = CoreSim(nc)
    for name, arr in maps[core].items():
        sim.tensor(name)[:] = arr
    sim.simulate()
    return np.array(sim.tensor("y"))


# revision 16
# speedup vs baseline: 4.7476x; 4.7476x over previous
"""ChannelWiseTCN Trainium2 Bass kernel.

Reference model: 4-layer TCN. Each layer: tied-kernel causal depthwise conv
(K=3, dilation 2^i) -> 1x1 pointwise conv -> BatchNorm(eval) -> ReLU ->
residual add -> ReLU. Final 1x1 conv to 1 channel.

Strategy:
  - Data-parallel over batch: 32 batches / 8 cores = 4 per core; small
    weights replicated.
  - Host-side folding: BN scale/shift folded into pointwise weights/bias.
    Depthwise tap scalar sc_w[i,k] folded into the pointwise weight too, so
    each layer becomes 3 matmuls (one per tap, shifted view of the input
    along the seq dim) accumulated in PSUM, plus bias+ReLU epilogue.
  - Activations stay SBUF-resident: x [64, 16+4096] per batch, h buffers
    [128, 16+4096]; the 16-col zero pad provides the causal-conv history
    (max lookback = (K-1)*d_max = 16).
"""

import os
import numpy as np
import ml_dtypes

import concourse.bass as bass
import concourse.tile as tile
from concourse import bacc, mybir
from concourse.bass_utils import run_bass_kernel_spmd

F32 = mybir.dt.float32
BF16 = mybir.dt.bfloat16
AF = mybir.ActivationFunctionType
ALU = mybir.AluOpType

# "f32": exact; "f32r": fp32 data, fast fp32r PE path; "bf16": weights +
# activations in bf16 (PSUM/bias/output stay fp32), ~3e-3 rel err, 2x PE.
PRECISION = "bf16"

B, CIN, H, L, K, NL = 32, 64, 128, 4096, 3, 4
EPS = 1e-5
NCORES = 8
BLOC = B // NCORES          # batches per core
PAD = 16                    # max causal lookback: (K-1) * 2^(NL-1) = 16
CH = 512                    # seq chunk (PSUM fp32 free-dim limit)
NCH = L // CH

LAST_RESULTS = None         # stash for test.py (exec_time_ns etc.)
REPLICAS = 1                # timing knob: run the whole TCN R times in one NEFF


def _preprocess(inputs):
    """Fold BN + tap scalars into matmul-ready weights (host, float64)."""
    f64 = {k: np.asarray(v, np.float64) for k, v in inputs.items()}
    s = f64["bn_g"] / np.sqrt(f64["bn_v"] + EPS)                 # (NL,H)
    bias = s * (f64["pw_b"] - f64["bn_m"]) + f64["bn_b"]         # (NL,H)

    sc = f64["sc_w"]                                             # (NL,K)
    W0 = s[0][:, None] * f64["pw_w0"]                            # (H,CIN)
    lhsT0 = np.stack([W0.T * sc[0, k] for k in range(K)])        # (K,CIN,H)
    lhsTr = np.stack([
        np.stack([(s[i][:, None] * f64["pw_w_rest"][i - 1]).T * sc[i, k]
                  for k in range(K)])
        for i in range(1, NL)
    ])                                                           # (NL-1,K,H,H)

    wdt = ml_dtypes.bfloat16 if PRECISION == "bf16" else np.float32
    return {
        "lhsT0": lhsT0.astype(wdt),
        "lhsTr": lhsTr.astype(wdt),
        "resT": np.ascontiguousarray(f64["res_w"].T).astype(wdt),   # (CIN,H)
        "outT": np.ascontiguousarray(f64["out_w"].T).astype(wdt),   # (H,1)
        "bias": bias[:, :, None].astype(np.float32),             # (NL,H,1)
        "res_b": f64["res_b"][:, None].astype(np.float32),       # (H,1)
        "out_b": f64["out_b"][:, None].astype(np.float32),       # (1,1)
    }


def _build():
    """Build the per-core Bass program. Returns nc."""
    nc = bacc.Bacc("TRN2", target_bir_lowering=False, debug=False)

    MDT = BF16 if PRECISION == "bf16" else F32   # matmul-operand storage dtype

    def mm(out, lhsT, rhs, start, stop):
        if PRECISION == "f32r":
            lhsT = lhsT.bitcast(mybir.dt.float32r)
            rhs = rhs.bitcast(mybir.dt.float32r)
        nc.tensor.matmul(out, lhsT, rhs, start=start, stop=stop)

    x_d = nc.dram_tensor("x", [BLOC, CIN, L], MDT, kind="ExternalInput").ap()
    lhsT0_d = nc.dram_tensor("lhsT0", [K, CIN, H], MDT, kind="ExternalInput").ap()
    lhsTr_d = nc.dram_tensor("lhsTr", [NL - 1, K, H, H], MDT, kind="ExternalInput").ap()
    resT_d = nc.dram_tensor("resT", [CIN, H], MDT, kind="ExternalInput").ap()
    outT_d = nc.dram_tensor("outT", [H, 1], MDT, kind="ExternalInput").ap()
    bias_d = nc.dram_tensor("bias", [NL, H, 1], F32, kind="ExternalInput").ap()
    res_b_d = nc.dram_tensor("res_b", [H, 1], F32, kind="ExternalInput").ap()
    out_b_d = nc.dram_tensor("out_b", [1, 1], F32, kind="ExternalInput").ap()
    y_d = nc.dram_tensor("y", [BLOC, 1, L], F32, kind="ExternalOutput").ap()

    with tile.TileContext(nc) as tc:
        with (
            tc.tile_pool(name="const", bufs=1) as cpool,
            tc.tile_pool(name="xp", bufs=2) as xpool,
            tc.tile_pool(name="hp", bufs=3) as hpool,
            tc.tile_pool(name="ap", bufs=4) as apool,
            tc.tile_pool(name="op", bufs=2) as opool,
            tc.tile_pool(name="psy", bufs=4, space="PSUM") as psy,
            tc.tile_pool(name="psr", bufs=2, space="PSUM") as psr,
            tc.tile_pool(name="pso", bufs=2, space="PSUM") as pso,
        ):
            # ---- load weights/biases once ----
            wt = []  # wt[i][k] lhsT tile
            row = []
            for kk in range(K):
                t = cpool.tile([CIN, H], MDT, name=f"w0t_{kk}")
                nc.sync.dma_start(t[:], lhsT0_d[kk])
                row.append(t)
            wt.append(row)
            for i in range(1, NL):
                row = []
                for kk in range(K):
                    t = cpool.tile([H, H], MDT, name=f"w{i}t_{kk}")
                    nc.sync.dma_start(t[:], lhsTr_d[i - 1, kk])
                    row.append(t)
                wt.append(row)
            wres = cpool.tile([CIN, H], MDT, name="wres")
            nc.sync.dma_start(wres[:], resT_d[:])
            wout = cpool.tile([H, 1], MDT, name="wout")
            nc.sync.dma_start(wout[:], outT_d[:])
            btile = []
            for i in range(NL):
                t = cpool.tile([H, 1], F32, name=f"bias_{i}")
                nc.sync.dma_start(t[:], bias_d[i])
                btile.append(t)
            rb = cpool.tile([H, 1], F32, name="res_b")
            nc.sync.dma_start(rb[:], res_b_d[:])
            ob = cpool.tile([1, 1], F32, name="out_b")
            nc.sync.dma_start(ob[:], out_b_d[:])

            # All-engine sync point: later instructions never carry waits on
            # the const-load DMA queues (walrus caps sync waits per inst).
            tc.strict_bb_all_engine_barrier()

            # ---- main loop ----
            for b in [bb for _ in range(REPLICAS) for bb in range(BLOC)]:
                xt = xpool.tile([CIN, PAD + L], MDT, name="xt", tag="xt")
                nc.vector.memset(xt[:, :PAD], 0.0)
                nc.sync.dma_start(xt[:, PAD:], x_d[b])

                hprev = xt
                for i in range(NL):
                    d = 1 << i
                    ht = hpool.tile([H, PAD + L], MDT, name=f"h{i}", tag="h")
                    nc.vector.memset(ht[:, :PAD], 0.0)
                    for j in range(NCH):
                        c0 = PAD + j * CH
                        yp = psy.tile([H, CH], F32, name="yp", tag="yp")
                        for kk in range(K):
                            off = (K - 1 - kk) * d
                            mm(yp[:], wt[i][kk][:],
                               hprev[:, c0 - off: c0 - off + CH],
                               start=(kk == 0), stop=(kk == K - 1))
                        at = apool.tile([H, CH], MDT, name="at", tag="at")
                        nc.scalar.activation(at[:], yp[:], AF.Relu, bias=btile[i][:])
                        dst = ht[:, c0: c0 + CH]
                        if i == 0:
                            rp = psr.tile([H, CH], F32, name="rp", tag="rp")
                            mm(rp[:], wres[:], xt[:, c0: c0 + CH],
                               start=True, stop=True)
                            nc.vector.scalar_tensor_tensor(
                                dst, rp[:], rb[:], at[:], ALU.add, ALU.add)
                            nc.vector.tensor_scalar_max(dst, dst, 0.0)
                        else:
                            # relu(relu(y)+h_prev) == relu(y)+h_prev: both
                            # operands are non-negative post-ReLU.
                            nc.vector.tensor_add(dst, at[:], hprev[:, c0: c0 + CH])
                    hprev = ht

                ot = opool.tile([1, L], F32, name="ot", tag="ot")
                for j in range(NCH):
                    c0 = PAD + j * CH
                    opsum = pso.tile([1, CH], F32, name="opsum", tag="opsum")
                    mm(opsum[:], wout[:], hprev[:, c0: c0 + CH],
                       start=True, stop=True)
                    nc.scalar.activation(ot[:, j * CH:(j + 1) * CH], opsum[:],
                                         AF.Identity, bias=ob[:])
                nc.sync.dma_start(y_d[b], ot[:])

    nc.compile()
    return nc


def _in_maps(inputs):
    w = _preprocess(inputs)
    xdt = ml_dtypes.bfloat16 if PRECISION == "bf16" else np.float32
    x = np.asarray(inputs["x"], np.float32).astype(xdt)
    maps = []
    for c in range(NCORES):
        m = dict(w)
        m["x"] = np.ascontiguousarray(x[c * BLOC:(c + 1) * BLOC])
        maps.append(m)
    return maps


def kernel(**inputs):
    global LAST_RESULTS
    os.environ.setdefault("BASS_NEVER_TRACE", "1")  # no NTFF hook in this env
    nc = _build()
    maps = _in_maps(inputs)
    res = run_bass_kernel_spmd(nc, maps, core_ids=list(range(NCORES)))
    LAST_RESULTS = res
    out = np.concatenate([r["y"] for r in res.results], axis=0)
    return out.astype(np.float32)


def simulate_core(inputs, core=0):
    """CoreSim numeric check of one core's shard (no hardware)."""
    from concourse.bass_interp import CoreSim
    nc = _build()
    maps = _in_maps(inputs)
    sim = CoreSim(nc)
    for name, arr in maps[core].items():
        sim.tensor(name)[:] = arr
    sim.simulate()
    return np.array(sim.tensor("y"))
